# revision 2
# baseline (speedup 1.0000x reference)
"""Trainium2 Bass kernel for nn_CustomModel_71725953843992 (pairwise-distance loss).

reference math:
    fw = feat * W                      # [C,R,D]
    distX = sum_c clamp0(n_c_i + n_c_j - 2 * fw_c @ fw_c.T)   # [R,R]
    dist  = distX with diag replaced by max(distX)
    loss  = sum(dist * S^2) + penalties(S, W)

Device strategy (8 cores, row-sharded):
    Core k owns rows [512k, 512k+512). Per [128,512] output tile it runs
    8 bf16 matmuls (one per channel, accumulating  -2*X_slab @ X_all^T in
    PSUM) plus one K=1 broadcast matmul adding the column norms n_j, then
    relu(psum + n_i) on ScalarE, and a fused multiply+row-reduce against
    S^2 (diagonal pre-zeroed on host) on VectorE.  Per-channel clamping is
    folded into one final relu: each channel's distance is >= 0 up to
    ~1e-9 rounding, so clamp(sum) == sum(clamp) to well below tolerance.
    The diagonal/max term and the scalar penalties are assembled on host
    from tiny per-core partials ([128,32] row-sums and row-maxes).
"""

import os
import sys

import numpy as np

for _p in ("/opt/trn_rl_repo", "/opt/trn_rl_repo/concourse"):
    if _p not in sys.path:
        sys.path.insert(0, _p)

import ml_dtypes

C, R, D = 8, 4096, 128
NCORES = 8
RS = R // NCORES      # 512 rows per core
NT = RS // 128        # 4 row-tiles per core
NJ = R // 512         # 8 column tiles of 512
BETA = 1.0

_compiled = None
LAST_RESULTS = None   # BassKernelResults of the most recent run (for test harness)


def _build():
    import concourse.bass as bass
    import concourse.mybir as mybir
    import concourse.tile as tile
    from concourse import bacc
    from concourse.bass import ts
    from contextlib import ExitStack

    f32 = mybir.dt.float32
    bf16 = mybir.dt.bfloat16

    nc = bacc.Bacc(
        "TRN2",
        target_bir_lowering=False,
        debug=False,
        enable_asserts=False,
        num_devices=NCORES,
    )

    fwt_d = nc.dram_tensor("fwt", [128, NJ, C, 512], bf16, kind="ExternalInput")
    lhsT_d = nc.dram_tensor("lhsT", [128, C, NT, 128], bf16, kind="ExternalInput")
    s2_d = nc.dram_tensor("s2", [NT, 128, R], f32, kind="ExternalInput")
    ncols_d = nc.dram_tensor("ncols", [1, R], bf16, kind="ExternalInput")
    nrows_d = nc.dram_tensor("nrows", [128, NT], f32, kind="ExternalInput")
    ones_d = nc.dram_tensor("ones", [1, 128], bf16, kind="ExternalInput")
    osum_d = nc.dram_tensor("out_sum", [128, NT * NJ], f32, kind="ExternalOutput")
    omax_d = nc.dram_tensor("out_max", [128, NT * NJ], f32, kind="ExternalOutput")

    with tile.TileContext(nc) as tc, ExitStack() as ctx:
        const = ctx.enter_context(tc.tile_pool(name="const", bufs=1))
        s2p = ctx.enter_context(tc.tile_pool(name="s2p", bufs=2))
        psum = ctx.enter_context(tc.tile_pool(name="psum", bufs=4, space="PSUM"))
        work = ctx.enter_context(tc.tile_pool(name="work", bufs=3))

        fwt_sb = const.tile([128, NJ, C, 512], bf16)
        for j in range(NJ):
            nc.sync.dma_start(fwt_sb[:, j], fwt_d.ap()[:, j])
        lhsT_sb = const.tile([128, C, NT, 128], bf16)
        nc.sync.dma_start(lhsT_sb[:], lhsT_d.ap()[:])
        ncols_sb = const.tile([1, R], bf16)
        nc.sync.dma_start(ncols_sb[:], ncols_d.ap()[:])
        nrows_sb = const.tile([128, NT], f32)
        nc.sync.dma_start(nrows_sb[:], nrows_d.ap()[:])
        ones_sb = const.tile([1, 128], bf16)
        nc.sync.dma_start(ones_sb[:], ones_d.ap()[:])
        acc_sb = const.tile([128, NT * NJ], f32)
        mx_sb = const.tile([128, NT * NJ], f32)

        for t in range(NT):
            s2_sb = s2p.tile([128, R], f32)
            nc.sync.dma_start(s2_sb[:], s2_d.ap()[t])
            for j in range(NJ):
                p = psum.tile([128, 512], f32)
                for c in range(C):
                    nc.tensor.matmul(
                        p[:],
                        lhsT_sb[:, c, t, :],
                        fwt_sb[:, j, c, :],
                        start=(c == 0),
                        stop=False,
                    )
                nc.tensor.matmul(
                    p[:],
                    ones_sb[:, :],
                    ncols_sb[:, ts(j, 512)],
                    start=False,
                    stop=True,
                )
                idx = t * NJ + j
                # sum path: (psum + n_i) * s2, row-summed.  No relu needed:
                # only diagonal entries can go (tiny) negative and s2's
                # diagonal is zeroed on host.
                wp = work.tile([128, 512], f32, tag="wp")
                nc.vector.scalar_tensor_tensor(
                    wp[:],
                    p[:],
                    nrows_sb[:, t : t + 1],
                    s2_sb[:, ts(j, 512)],
                    op0=mybir.AluOpType.add,
                    op1=mybir.AluOpType.mult,
                    accum_out=acc_sb[:, idx : idx + 1],
                )
                # max path: relu(psum + n_i) matches the reference's clamp.
                dist = work.tile([128, 512], f32, tag="dist")
                nc.scalar.activation(
                    dist[:],
                    p[:],
                    mybir.ActivationFunctionType.Relu,
                    bias=nrows_sb[:, t : t + 1],
                )
                nc.vector.tensor_reduce(
                    mx_sb[:, idx : idx + 1],
                    dist[:],
                    axis=mybir.AxisListType.X,
                    op=mybir.AluOpType.max,
                )
        nc.sync.dma_start(osum_d.ap()[:], acc_sb[:])
        nc.sync.dma_start(omax_d.ap()[:], mx_sb[:])

    nc.compile()
    return nc


def _get_compiled():
    global _compiled
    if _compiled is None:
        _compiled = _build()
    return _compiled


def kernel(feat: np.ndarray, S: np.ndarray, W: np.ndarray):
    global LAST_RESULTS
    from concourse.bass_utils import run_bass_kernel_spmd

    feat = np.asarray(feat, np.float32)
    S = np.asarray(S, np.float32)
    W = np.asarray(W, np.float32)

    # ---- host prep ----
    fw = feat * W                                   # [C,R,D] f32
    fwt = np.ascontiguousarray(fw.transpose(0, 2, 1))  # [C,D,R]
    n_tot = (fw.astype(np.float64) ** 2).sum(axis=(0, 2)).astype(np.float32)  # [R]

    # fwt dram layout [128(d), NJ, C, 512]: [d, j, c, n] = fwt[c, d, j*512+n]
    fwt_host = np.ascontiguousarray(
        fwt.reshape(C, 128, NJ, 512).transpose(1, 2, 0, 3)
    ).astype(ml_dtypes.bfloat16)

    S2 = (S * S).astype(np.float32)
    np.fill_diagonal(S2, 0.0)

    ncols_host = n_tot.reshape(1, R).astype(ml_dtypes.bfloat16)
    ones_host = np.ones((1, 128), ml_dtypes.bfloat16)

    in_maps = []
    for k in range(NCORES):
        r0 = k * RS
        # lhsT dram layout [128(d), C, NT, 128(m)] = -2*fwt[c, d, r0+t*128+m]
        lhsT_host = np.ascontiguousarray(
            (-2.0 * fwt[:, :, r0 : r0 + RS]).reshape(C, 128, NT, 128).transpose(1, 0, 2, 3)
        ).astype(ml_dtypes.bfloat16)
        s2_host = np.ascontiguousarray(S2[r0 : r0 + RS].reshape(NT, 128, R))
        nrows_host = np.ascontiguousarray(n_tot[r0 : r0 + RS].reshape(NT, 128).T)
        in_maps.append(
            {
                "fwt": fwt_host,
                "lhsT": lhsT_host,
                "s2": s2_host,
                "ncols": ncols_host,
                "nrows": nrows_host,
                "ones": ones_host,
            }
        )

    nc = _get_compiled()
    res = run_bass_kernel_spmd(nc, in_maps, core_ids=list(range(NCORES)))
    LAST_RESULTS = res

    # ---- host assembly ----
    tot = 0.0
    dmax = -np.inf
    for k in range(NCORES):
        tot += float(np.asarray(res.results[k]["out_sum"], np.float64).sum())
        dmax = max(dmax, float(np.asarray(res.results[k]["out_max"]).max()))

    sdiag2 = float((np.diag(S).astype(np.float64) ** 2).sum())
    dist_S = tot + dmax * sdiag2

    W2 = W[:, 0, :]
    sum1_W = 100.0 * abs(float(W2.astype(np.float64).sum()) - W2.shape[1])
    sum1_S = 100.0 * abs(float(S.astype(np.float64).sum()) - R)
    pneg = float(np.where(S < 0, S, 0).astype(np.float64).sum())
    ppos = float(np.where(S > 1, S - 1, 0).astype(np.float64).sum())
    penalty = BETA * (-pneg + ppos)

    loss = np.float32(dist_S + BETA * (penalty + sum1_W + sum1_S))
    return (np.array(loss, np.float32), S, W)
